# revision 15
# baseline (speedup 1.0000x reference)
"""GroupConnected segment-reduce kernel for 8x Trainium2 NeuronCores.

Computes out[b, g] = sum_k x[b, idx[g, k]] * w[g, k] for
B=8192, F=8192, G=2048, K=8 (f32), sharding batch B across 8 cores.

Per-core pipeline, software-pipelined over batch stages so the
transpose (HWDGE DMA + PE) of stage s+1 overlaps the gather+reduce
(SWDGE DMA + PE) of stage s:
  Phase T(s): transpose x_local[:, stage s] -> DRAM scratch xT_s [F, bs]
              (PE transpose 128x128 blocks into a 4-block PSUM strip,
               one wide DVE copy per strip, large-tile DMA both ways).
  Phase G(s): per g-tile: one indirect DMA gathers all 8 k-slot row sets
              (128 rows x 2KB each) from xT_s, then 8 diag(w) matmuls
              accumulate over k in PSUM -> outT[gtile, stage].
Host transposes outT [G, Bc] back to [Bc, G] per core.
"""

import numpy as np

import concourse.bacc as bacc
import concourse.tile as tile
from concourse import bass, mybir
from concourse.bass_utils import run_bass_kernel_spmd
from concourse.masks import make_identity

P = 128

# Problem shape (hardcoded per contract).
B, F, G, K = 8192, 8192, 2048, 8
N_CORES = 8
BC = B // N_CORES  # 1024 batch rows per core

_compiled = {}


def build_program(Bc=BC, Fdim=F, Gdim=G, Kdim=K, panel_f=512, n_stages=2):
    """Build the SPMD Bass program (same program on every core)."""
    bs = Bc // n_stages                 # batch cols per stage
    assert bs % P == 0 and bs <= 512
    assert Fdim % panel_f == 0 and panel_f % P == 0 and Gdim % P == 0
    n_gt = Gdim // P                    # g-tiles
    n_pan = Fdim // panel_f             # column panels
    n_sub = panel_f // P                # 128-wide f sub-chunks per panel
    n_bbs = bs // P                     # b-blocks per stage

    nc = bacc.Bacc("TRN2", target_bir_lowering=False, debug=False)

    x_in = nc.dram_tensor("x_local", [Bc, Fdim], mybir.dt.float32,
                          kind="ExternalInput").ap()
    w_in = nc.dram_tensor("w_in", [Gdim, Kdim], mybir.dt.float32,
                          kind="ExternalInput").ap()
    gidx_in = nc.dram_tensor("gidx", [P, n_gt * Kdim], mybir.dt.int32,
                             kind="ExternalInput").ap()
    out_t = nc.dram_tensor("outT", [Gdim, Bc], mybir.dt.float32,
                           kind="ExternalOutput").ap()

    with tile.TileContext(nc) as tc:
        with tc.tile_pool(name="const", bufs=1) as cpool, \
             tc.tile_pool(name="xtd", bufs=1, space="DRAM") as xtpool, \
             tc.tile_pool(name="tload", bufs=6) as lpool, \
             tc.tile_pool(name="tpsum", bufs=2, space="PSUM") as tpsum, \
             tc.tile_pool(name="tpanel", bufs=2) as ppool, \
             tc.tile_pool(name="gath", bufs=4) as apool, \
             tc.tile_pool(name="gout", bufs=2) as opool:

            # xT stage scratch as DRAM *pool tiles* so Tile's access
            # tracker serializes the phase-G gathers behind the phase-T
            # writes with real DMA-completion semaphores.
            xTs = [xtpool.tile([Fdim, bs], dtype=mybir.dt.float32,
                               name=f"xT{s}", tag=f"xT{s}")
                   for s in range(n_stages)]

            ident = cpool.tile([P, P], dtype=mybir.dt.float32)
            make_identity(nc, ident[:])

            # constants for phase G (stage-independent)
            w_sb = cpool.tile([P, n_gt * Kdim], dtype=mybir.dt.float32)
            for t in range(n_gt):
                nc.sync.dma_start(out=w_sb[:, t * Kdim:(t + 1) * Kdim],
                                  in_=w_in[t * P:(t + 1) * P, :])
            gidx_sb = cpool.tile([P, n_gt * Kdim], dtype=mybir.dt.int32)
            nc.sync.dma_start(out=gidx_sb[:], in_=gidx_in[:])

            def phase_t_panel(s, p):
                """Transpose one column panel of stage s into xTs[s]."""
                if True:
                    strip = tpsum.tile([P, n_sub * n_bbs * P],
                                       dtype=mybir.dt.float32, space="PSUM",
                                       name="strip", tag="strip")
                    panels = [ppool.tile([P, bs], dtype=mybir.dt.float32,
                                         name=f"xtp{c}", tag=f"xtp{c}")
                              for c in range(n_sub)]
                    for j in range(n_bbs):
                        xb = lpool.tile([P, panel_f], dtype=mybir.dt.float32,
                                        name="xb", tag="xb")
                        nc.sync.dma_start(
                            out=xb[:],
                            in_=x_in[(s * n_bbs + j) * P:
                                     (s * n_bbs + j + 1) * P,
                                     p * panel_f:(p + 1) * panel_f])
                        for c in range(n_sub):
                            col = (c * n_bbs + j) * P
                            nc.tensor.transpose(
                                out=strip[:, col:col + P],
                                in_=xb[:, c * P:(c + 1) * P],
                                identity=ident[:])
                    for c in range(n_sub):
                        nc.vector.tensor_copy(
                            out=panels[c][:],
                            in_=strip[:, c * n_bbs * P:(c + 1) * n_bbs * P])
                        row0 = p * panel_f + c * P
                        nc.sync.dma_start(out=xTs[s][row0:row0 + P, :],
                                          in_=panels[c][:])

            def phase_g_tile(s, t):
                """Gather + reduce one g-tile of stage s."""
                if True:
                    a = apool.tile([P, Kdim * bs], dtype=mybir.dt.float32,
                                   name="agath", tag="agath")
                    for k in range(Kdim):
                        # NB: multi-offset indirect DMA miscomputes on HW
                        # (verified) — one gather per k-slot.
                        nc.gpsimd.indirect_dma_start(
                            out=a[:, k * bs:(k + 1) * bs], out_offset=None,
                            in_=xTs[s][:, :],
                            in_offset=bass.IndirectOffsetOnAxis(
                                ap=gidx_sb[:, t * Kdim + k:t * Kdim + k + 1],
                                axis=0))
                    # per-partition scale by w[g, k] on the (idle) ACT engine
                    for k in range(Kdim):
                        nc.scalar.activation(
                            out=a[:, k * bs:(k + 1) * bs],
                            in_=a[:, k * bs:(k + 1) * bs],
                            func=mybir.ActivationFunctionType.Copy,
                            scale=w_sb[:, t * Kdim + k:t * Kdim + k + 1])
                    # K-reduce: in-place add tree on DVE (wide contiguous ops)
                    width = Kdim * bs
                    while width > 2 * bs:
                        width //= 2
                        nc.vector.tensor_add(out=a[:, :width],
                                             in0=a[:, :width],
                                             in1=a[:, width:2 * width])
                    osb = opool.tile([P, bs], dtype=mybir.dt.float32,
                                     name="osb", tag="osb")
                    nc.vector.tensor_add(out=osb[:], in0=a[:, :bs],
                                         in1=a[:, bs:2 * bs])
                    nc.sync.dma_start(
                        out=out_t[t * P:(t + 1) * P, s * bs:(s + 1) * bs],
                        in_=osb[:])

            # Software pipeline: emit alternating (next-stage transpose
            # panel, current-stage gather tile) so every engine's
            # in-order stream interleaves both phases and the PE never
            # idles long enough to re-throttle.
            units = []   # list of ("t", s, p) / ("g", s, t)
            for p in range(n_pan):
                units.append(("t", 0, p))
            for s in range(n_stages):
                nxt = [("t", s + 1, p) for p in range(n_pan)] \
                    if s + 1 < n_stages else []
                cur = [("g", s, t) for t in range(n_gt)]
                k = max(len(nxt), len(cur))
                for i in range(k):
                    if i < len(nxt):
                        units.append(nxt[i])
                    if i < len(cur):
                        units.append(cur[i])
            for kind, s, i in units:
                if kind == "t":
                    phase_t_panel(s, i)
                else:
                    phase_g_tile(s, i)

    nc.compile()
    return nc


def _get_program():
    if "full" not in _compiled:
        _compiled["full"] = build_program()
    return _compiled["full"]


def kernel(x, group_idx, w, _trace=False):
    x = np.ascontiguousarray(np.asarray(x), dtype=np.float32)
    w_np = np.ascontiguousarray(np.asarray(w), dtype=np.float32)
    gi = np.asarray(group_idx).astype(np.int64)

    # gather-index table: column t*K+k holds idx[t*128:(t+1)*128, k] (int32)
    n_gt = G // P
    tbl = np.empty((P, n_gt * K), dtype=np.int32)
    for t in range(n_gt):
        tbl[:, t * K:(t + 1) * K] = gi[t * P:(t + 1) * P, :].astype(np.int32)

    nc = _get_program()
    in_maps = [
        {"x_local": x[i * BC:(i + 1) * BC, :], "w_in": w_np, "gidx": tbl}
        for i in range(N_CORES)
    ]
    res = run_bass_kernel_spmd(nc, in_maps, list(range(N_CORES)),
                               trace=_trace)
    out = np.empty((B, G), dtype=np.float32)
    for i in range(N_CORES):
        out[i * BC:(i + 1) * BC, :] = res.results[i]["outT"].T
    if _trace:
        return out, res
    return out


# revision 18
# speedup vs baseline: 1.1392x; 1.1392x over previous
"""GroupConnected segment-reduce kernel for 8x Trainium2 NeuronCores.

Computes out[b, g] = sum_k x[b, idx[g, k]] * w[g, k] for
B=8192, F=8192, G=2048, K=8 (f32), sharding batch B across 8 cores.

Per-core pipeline, software-pipelined over batch stages so the
transpose (HWDGE DMA + PE) of stage s+1 overlaps the gather+reduce
(SWDGE DMA + PE) of stage s:
  Phase T(s): transpose x_local[:, stage s] -> DRAM scratch xT_s [F, bs]
              (PE transpose 128x128 blocks into a 4-block PSUM strip,
               one wide DVE copy per strip, large-tile DMA both ways).
  Phase G(s): per g-tile: one indirect DMA gathers all 8 k-slot row sets
              (128 rows x 2KB each) from xT_s, then 8 diag(w) matmuls
              accumulate over k in PSUM -> outT[gtile, stage].
Host transposes outT [G, Bc] back to [Bc, G] per core.
"""

import numpy as np

import concourse.bacc as bacc
import concourse.tile as tile
from concourse import bass, mybir
from concourse.bass_utils import run_bass_kernel_spmd
from concourse.masks import make_identity

P = 128

# Problem shape (hardcoded per contract).
B, F, G, K = 8192, 8192, 2048, 8
N_CORES = 8
BC = B // N_CORES  # 1024 batch rows per core

_compiled = {}


def build_program(Bc=BC, Fdim=F, Gdim=G, Kdim=K, panel_f=512, n_stages=2):
    """Build the SPMD Bass program (same program on every core)."""
    bs = Bc // n_stages                 # batch cols per stage
    assert bs % P == 0
    assert Fdim % panel_f == 0 and panel_f % P == 0 and Gdim % P == 0
    n_gt = Gdim // P                    # g-tiles
    n_pan = Fdim // panel_f             # column panels
    n_sub = panel_f // P                # 128-wide f sub-chunks per panel
    n_bbs = bs // P                     # b-blocks per stage
    quad = min(4, n_bbs)                # b-blocks per PSUM strip (bank cap)
    assert n_bbs % quad == 0

    nc = bacc.Bacc("TRN2", target_bir_lowering=False, debug=False)

    x_in = nc.dram_tensor("x_local", [Bc, Fdim], mybir.dt.float32,
                          kind="ExternalInput").ap()
    w_in = nc.dram_tensor("w_in", [Gdim, Kdim], mybir.dt.float32,
                          kind="ExternalInput").ap()
    gidx_in = nc.dram_tensor("gidx", [P, n_gt * Kdim], mybir.dt.int32,
                             kind="ExternalInput").ap()
    out_t = nc.dram_tensor("outT", [Gdim, Bc], mybir.dt.float32,
                           kind="ExternalOutput").ap()

    with tile.TileContext(nc) as tc:
        with tc.tile_pool(name="const", bufs=1) as cpool, \
             tc.tile_pool(name="xtd", bufs=1, space="DRAM") as xtpool, \
             tc.tile_pool(name="tload", bufs=6) as lpool, \
             tc.tile_pool(name="tpsum", bufs=2, space="PSUM") as tpsum, \
             tc.tile_pool(name="tpanel", bufs=2) as ppool, \
             tc.tile_pool(name="gath", bufs=4) as apool, \
             tc.tile_pool(name="gout", bufs=2) as opool:

            # xT stage scratch as DRAM *pool tiles* so Tile's access
            # tracker serializes the phase-G gathers behind the phase-T
            # writes with real DMA-completion semaphores.
            xTs = [xtpool.tile([Fdim, bs], dtype=mybir.dt.float32,
                               name=f"xT{s}", tag=f"xT{s}")
                   for s in range(n_stages)]

            ident = cpool.tile([P, P], dtype=mybir.dt.float32)
            make_identity(nc, ident[:])

            # constants for phase G (stage-independent)
            w_sb = cpool.tile([P, n_gt * Kdim], dtype=mybir.dt.float32)
            for t in range(n_gt):
                nc.sync.dma_start(out=w_sb[:, t * Kdim:(t + 1) * Kdim],
                                  in_=w_in[t * P:(t + 1) * P, :])
            gidx_sb = cpool.tile([P, n_gt * Kdim], dtype=mybir.dt.int32)
            nc.sync.dma_start(out=gidx_sb[:], in_=gidx_in[:])

            def phase_t_panel(s, p):
                """Transpose one column panel of stage s into xTs[s]."""
                panels = [ppool.tile([P, bs], dtype=mybir.dt.float32,
                                     name=f"xtp{c}", tag=f"xtp{c}")
                          for c in range(n_sub)]
                for q in range(n_bbs // quad):
                    strip = tpsum.tile([P, n_sub * quad * P],
                                       dtype=mybir.dt.float32, space="PSUM",
                                       name="strip", tag="strip")
                    for jj in range(quad):
                        j = q * quad + jj
                        xb = lpool.tile([P, panel_f], dtype=mybir.dt.float32,
                                        name="xb", tag="xb")
                        nc.sync.dma_start(
                            out=xb[:],
                            in_=x_in[(s * n_bbs + j) * P:
                                     (s * n_bbs + j + 1) * P,
                                     p * panel_f:(p + 1) * panel_f])
                        for c in range(n_sub):
                            col = (c * quad + jj) * P
                            nc.tensor.transpose(
                                out=strip[:, col:col + P],
                                in_=xb[:, c * P:(c + 1) * P],
                                identity=ident[:])
                    for c in range(n_sub):
                        nc.vector.tensor_copy(
                            out=panels[c][:, q * quad * P:(q + 1) * quad * P],
                            in_=strip[:, c * quad * P:(c + 1) * quad * P])
                for c in range(n_sub):
                    row0 = p * panel_f + c * P
                    nc.sync.dma_start(out=xTs[s][row0:row0 + P, :],
                                      in_=panels[c][:])

            def phase_g_tile(s, t):
                """Gather + reduce one g-tile of stage s."""
                if True:
                    a = apool.tile([P, Kdim * bs], dtype=mybir.dt.float32,
                                   name="agath", tag="agath")
                    for k in range(Kdim):
                        # NB: multi-offset indirect DMA miscomputes on HW
                        # (verified) — one gather per k-slot.
                        nc.gpsimd.indirect_dma_start(
                            out=a[:, k * bs:(k + 1) * bs], out_offset=None,
                            in_=xTs[s][:, :],
                            in_offset=bass.IndirectOffsetOnAxis(
                                ap=gidx_sb[:, t * Kdim + k:t * Kdim + k + 1],
                                axis=0))
                    # per-partition scale by w[g, k] on the (idle) ACT engine
                    for k in range(Kdim):
                        nc.scalar.activation(
                            out=a[:, k * bs:(k + 1) * bs],
                            in_=a[:, k * bs:(k + 1) * bs],
                            func=mybir.ActivationFunctionType.Copy,
                            scale=w_sb[:, t * Kdim + k:t * Kdim + k + 1])
                    # K-reduce: in-place add tree on DVE (wide contiguous ops)
                    width = Kdim * bs
                    while width > 2 * bs:
                        width //= 2
                        nc.vector.tensor_add(out=a[:, :width],
                                             in0=a[:, :width],
                                             in1=a[:, width:2 * width])
                    osb = opool.tile([P, bs], dtype=mybir.dt.float32,
                                     name="osb", tag="osb")
                    nc.vector.tensor_add(out=osb[:], in0=a[:, :bs],
                                         in1=a[:, bs:2 * bs])
                    nc.sync.dma_start(
                        out=out_t[t * P:(t + 1) * P, s * bs:(s + 1) * bs],
                        in_=osb[:])

            # Software pipeline: emit alternating (next-stage transpose
            # panel, current-stage gather tile) so every engine's
            # in-order stream interleaves both phases and the PE never
            # idles long enough to re-throttle.
            units = []   # list of ("t", s, p) / ("g", s, t)
            for p in range(n_pan):
                units.append(("t", 0, p))
            for s in range(n_stages):
                nxt = [("t", s + 1, p) for p in range(n_pan)] \
                    if s + 1 < n_stages else []
                cur = [("g", s, t) for t in range(n_gt)]
                k = max(len(nxt), len(cur))
                for i in range(k):
                    if i < len(nxt):
                        units.append(nxt[i])
                    if i < len(cur):
                        units.append(cur[i])
            for kind, s, i in units:
                if kind == "t":
                    phase_t_panel(s, i)
                else:
                    phase_g_tile(s, i)

    nc.compile()
    return nc


def _get_program():
    if "full" not in _compiled:
        _compiled["full"] = build_program(n_stages=1)
    return _compiled["full"]


def kernel(x, group_idx, w, _trace=False):
    x = np.ascontiguousarray(np.asarray(x), dtype=np.float32)
    w_np = np.ascontiguousarray(np.asarray(w), dtype=np.float32)
    gi = np.asarray(group_idx).astype(np.int64)

    # gather-index table: column t*K+k holds idx[t*128:(t+1)*128, k] (int32)
    n_gt = G // P
    tbl = np.empty((P, n_gt * K), dtype=np.int32)
    for t in range(n_gt):
        tbl[:, t * K:(t + 1) * K] = gi[t * P:(t + 1) * P, :].astype(np.int32)

    nc = _get_program()
    in_maps = [
        {"x_local": x[i * BC:(i + 1) * BC, :], "w_in": w_np, "gidx": tbl}
        for i in range(N_CORES)
    ]
    res = run_bass_kernel_spmd(nc, in_maps, list(range(N_CORES)),
                               trace=_trace)
    out = np.empty((B, G), dtype=np.float32)
    for i in range(N_CORES):
        out[i * BC:(i + 1) * BC, :] = res.results[i]["outT"].T
    if _trace:
        return out, res
    return out


# revision 19
# speedup vs baseline: 1.2033x; 1.0562x over previous
"""GroupConnected segment-reduce kernel for 8x Trainium2 NeuronCores.

Computes out[b, g] = sum_k x[b, idx[g, k]] * w[g, k] for
B=8192, F=8192, G=2048, K=8 (f32), sharding batch B across 8 cores.

Per-core pipeline, software-pipelined over batch stages so the
transpose (HWDGE DMA + PE) of stage s+1 overlaps the gather+reduce
(SWDGE DMA + PE) of stage s:
  Phase T(s): transpose x_local[:, stage s] -> DRAM scratch xT_s [F, bs]
              (PE transpose 128x128 blocks into a 4-block PSUM strip,
               one wide DVE copy per strip, large-tile DMA both ways).
  Phase G(s): per g-tile: one indirect DMA gathers all 8 k-slot row sets
              (128 rows x 2KB each) from xT_s, then 8 diag(w) matmuls
              accumulate over k in PSUM -> outT[gtile, stage].
Host transposes outT [G, Bc] back to [Bc, G] per core.
"""

import numpy as np

import concourse.bacc as bacc
import concourse.tile as tile
from concourse import bass, mybir
from concourse.bass_utils import run_bass_kernel_spmd
from concourse.masks import make_identity

P = 128

# Problem shape (hardcoded per contract).
B, F, G, K = 8192, 8192, 2048, 8
N_CORES = 8
BC = B // N_CORES  # 1024 batch rows per core

_compiled = {}


def build_program(Bc=BC, Fdim=F, Gdim=G, Kdim=K, panel_f=512, n_stages=2):
    """Build the SPMD Bass program (same program on every core)."""
    bs = Bc // n_stages                 # batch cols per stage
    assert bs % P == 0
    assert Fdim % panel_f == 0 and panel_f % P == 0 and Gdim % P == 0
    n_gt = Gdim // P                    # g-tiles
    n_pan = Fdim // panel_f             # column panels
    n_sub = panel_f // P                # 128-wide f sub-chunks per panel
    n_bbs = bs // P                     # b-blocks per stage
    quad = min(4, n_bbs)                # b-blocks per PSUM strip (bank cap)
    assert n_bbs % quad == 0

    nc = bacc.Bacc("TRN2", target_bir_lowering=False, debug=False)

    x_in = nc.dram_tensor("x_local", [Bc, Fdim], mybir.dt.float32,
                          kind="ExternalInput").ap()
    w_in = nc.dram_tensor("w_in", [Gdim, Kdim], mybir.dt.float32,
                          kind="ExternalInput").ap()
    gidx_in = nc.dram_tensor("gidx", [P, n_gt * Kdim], mybir.dt.int32,
                             kind="ExternalInput").ap()
    out_t = nc.dram_tensor("outT", [Gdim, Bc], mybir.dt.float32,
                           kind="ExternalOutput").ap()

    with tile.TileContext(nc) as tc:
        with tc.tile_pool(name="const", bufs=1) as cpool, \
             tc.tile_pool(name="xtd", bufs=1, space="DRAM") as xtpool, \
             tc.tile_pool(name="tload", bufs=8) as lpool, \
             tc.tile_pool(name="tpsum", bufs=2, space="PSUM") as tpsum, \
             tc.tile_pool(name="tpanel", bufs=2) as ppool, \
             tc.tile_pool(name="gath", bufs=4) as apool, \
             tc.tile_pool(name="gout", bufs=2) as opool:

            # xT stage scratch as DRAM *pool tiles* so Tile's access
            # tracker serializes the phase-G gathers behind the phase-T
            # writes with real DMA-completion semaphores.
            xTs = [xtpool.tile([Fdim, bs], dtype=mybir.dt.float32,
                               name=f"xT{s}", tag=f"xT{s}")
                   for s in range(n_stages)]

            ident = cpool.tile([P, P], dtype=mybir.dt.float32)
            make_identity(nc, ident[:])

            # constants for phase G (stage-independent)
            w_sb = cpool.tile([P, n_gt * Kdim], dtype=mybir.dt.float32)
            for t in range(n_gt):
                nc.sync.dma_start(out=w_sb[:, t * Kdim:(t + 1) * Kdim],
                                  in_=w_in[t * P:(t + 1) * P, :])
            gidx_sb = cpool.tile([P, n_gt * Kdim], dtype=mybir.dt.int32)
            nc.sync.dma_start(out=gidx_sb[:], in_=gidx_in[:])

            def phase_t_panel(s, p):
                """Transpose one column panel of stage s into xTs[s]."""
                panels = [ppool.tile([P, bs], dtype=mybir.dt.float32,
                                     name=f"xtp{c}", tag=f"xtp{c}")
                          for c in range(n_sub)]
                for q in range(n_bbs // quad):
                    strip = tpsum.tile([P, n_sub * quad * P],
                                       dtype=mybir.dt.float32, space="PSUM",
                                       name="strip", tag="strip")
                    for jj in range(quad):
                        j = q * quad + jj
                        xb = lpool.tile([P, panel_f], dtype=mybir.dt.float32,
                                        name="xb", tag="xb")
                        nc.sync.dma_start(
                            out=xb[:],
                            in_=x_in[(s * n_bbs + j) * P:
                                     (s * n_bbs + j + 1) * P,
                                     p * panel_f:(p + 1) * panel_f])
                        for c in range(n_sub):
                            col = (c * quad + jj) * P
                            nc.tensor.transpose(
                                out=strip[:, col:col + P],
                                in_=xb[:, c * P:(c + 1) * P],
                                identity=ident[:])
                    for c in range(n_sub):
                        nc.vector.tensor_copy(
                            out=panels[c][:, q * quad * P:(q + 1) * quad * P],
                            in_=strip[:, c * quad * P:(c + 1) * quad * P])
                for c in range(n_sub):
                    row0 = p * panel_f + c * P
                    nc.scalar.dma_start(out=xTs[s][row0:row0 + P, :],
                                        in_=panels[c][:])

            def phase_g_tile(s, t):
                """Gather + reduce one g-tile of stage s."""
                if True:
                    a = apool.tile([P, Kdim * bs], dtype=mybir.dt.float32,
                                   name="agath", tag="agath")
                    for k in range(Kdim):
                        # NB: multi-offset indirect DMA miscomputes on HW
                        # (verified) — one gather per k-slot.
                        nc.gpsimd.indirect_dma_start(
                            out=a[:, k * bs:(k + 1) * bs], out_offset=None,
                            in_=xTs[s][:, :],
                            in_offset=bass.IndirectOffsetOnAxis(
                                ap=gidx_sb[:, t * Kdim + k:t * Kdim + k + 1],
                                axis=0))
                    # per-partition scale by w[g, k] on the (idle) ACT engine
                    for k in range(Kdim):
                        nc.scalar.activation(
                            out=a[:, k * bs:(k + 1) * bs],
                            in_=a[:, k * bs:(k + 1) * bs],
                            func=mybir.ActivationFunctionType.Copy,
                            scale=w_sb[:, t * Kdim + k:t * Kdim + k + 1])
                    # K-reduce: in-place add tree on DVE (wide contiguous ops)
                    width = Kdim * bs
                    while width > 2 * bs:
                        width //= 2
                        nc.vector.tensor_add(out=a[:, :width],
                                             in0=a[:, :width],
                                             in1=a[:, width:2 * width])
                    osb = opool.tile([P, bs], dtype=mybir.dt.float32,
                                     name="osb", tag="osb")
                    nc.vector.tensor_add(out=osb[:], in0=a[:, :bs],
                                         in1=a[:, bs:2 * bs])
                    nc.sync.dma_start(
                        out=out_t[t * P:(t + 1) * P, s * bs:(s + 1) * bs],
                        in_=osb[:])

            # Software pipeline: emit alternating (next-stage transpose
            # panel, current-stage gather tile) so every engine's
            # in-order stream interleaves both phases and the PE never
            # idles long enough to re-throttle.
            units = []   # list of ("t", s, p) / ("g", s, t)
            for p in range(n_pan):
                units.append(("t", 0, p))
            for s in range(n_stages):
                nxt = [("t", s + 1, p) for p in range(n_pan)] \
                    if s + 1 < n_stages else []
                cur = [("g", s, t) for t in range(n_gt)]
                k = max(len(nxt), len(cur))
                for i in range(k):
                    if i < len(nxt):
                        units.append(nxt[i])
                    if i < len(cur):
                        units.append(cur[i])
            for kind, s, i in units:
                if kind == "t":
                    phase_t_panel(s, i)
                else:
                    phase_g_tile(s, i)

    nc.compile()
    return nc


def _get_program():
    if "full" not in _compiled:
        _compiled["full"] = build_program(n_stages=1)
    return _compiled["full"]


def kernel(x, group_idx, w, _trace=False):
    x = np.ascontiguousarray(np.asarray(x), dtype=np.float32)
    w_np = np.ascontiguousarray(np.asarray(w), dtype=np.float32)
    gi = np.asarray(group_idx).astype(np.int64)

    # gather-index table: column t*K+k holds idx[t*128:(t+1)*128, k] (int32)
    n_gt = G // P
    tbl = np.empty((P, n_gt * K), dtype=np.int32)
    for t in range(n_gt):
        tbl[:, t * K:(t + 1) * K] = gi[t * P:(t + 1) * P, :].astype(np.int32)

    nc = _get_program()
    in_maps = [
        {"x_local": x[i * BC:(i + 1) * BC, :], "w_in": w_np, "gidx": tbl}
        for i in range(N_CORES)
    ]
    res = run_bass_kernel_spmd(nc, in_maps, list(range(N_CORES)),
                               trace=_trace)
    out = np.empty((B, G), dtype=np.float32)
    for i in range(N_CORES):
        out[i * BC:(i + 1) * BC, :] = res.results[i]["outT"].T
    if _trace:
        return out, res
    return out
